# revision 17
# baseline (speedup 1.0000x reference)
"""Trainium2 Bass kernel for nn_CrossAxialMultiAttention (tied cross-axial attention).

Math (reference):
    q = x @ Wq.T + bq ; k = y @ Wk.T + bk ; v = y @ Wv.T + bv   (per axial slice m)
    qt = q.sum(m) ; kt = k.sum(m)                                (tied sum over M)
    w = softmax(qt @ kt.T / sqrt(D*M))                           (shared over m)
    out = (w @ v) @ Wp.T + bp ;  a = w[batch 0]

Key algebraic identity exploited: the Linear is affine, so
    qt = (x.sum(m)) @ Wq.T + M*bq  and  kt = (y.sum(m)) @ Wk.T + M*bk,
which removes a 32x redundancy in the Q/K projections.

Distribution over 8 cores:
  - The axial M dim (32) is sharded 4-per-core; V/PV/output-projection are local.
  - Per-core partial sums of x and y over its M-shard are projected through
    Wq/Wk, then a ReduceScatter (head-major layout) gives core i the completed
    qt/kt for head i.  Each core computes scores+softmax for its own head
    (both batches), then an AllGather of the normalized weights (fp16)
    replicates w to all cores for the PV stage.
  - Everything on device is laid out feature-major ([channels, seq] on
    [partitions, free]) so matmuls contract along partitions; the host
    pre-transposes inputs and post-transposes outputs (pure layout work).
"""

import os
import sys
from contextlib import ExitStack

import numpy as np

KDEBUG = os.environ.get("KDEBUG", "0") == "1"

sys.path.insert(0, "/opt/trn_rl_repo")

B, M, T, C = 2, 32, 512, 256
NH, D = 8, 32
N_CORES = 8
MLOC = M // N_CORES          # 4 axial slices per core
NCI = C // 128               # 2 k-tiles of the channel dim
ST = T // 128                # 4 sequence blocks of 128

_CACHE = {}


def _build():
    import concourse.bacc as bacc
    import concourse.tile as tile
    from concourse import mybir
    from concourse.bass import _add_dep_helper

    F32 = mybir.dt.float32
    F16 = mybir.dt.float16
    BF16 = mybir.dt.bfloat16
    AF = mybir.ActivationFunctionType
    ALU = mybir.AluOpType

    nc = bacc.Bacc("TRN2", target_bir_lowering=False, debug=False,
                   num_devices=N_CORES)

    x_sh = nc.dram_tensor("x_sh", [B, MLOC, C, T], F32, kind="ExternalInput").ap()
    y_sh = nc.dram_tensor("y_sh", [B, MLOC, C, T], F32, kind="ExternalInput").ap()
    wqT = nc.dram_tensor("wqT", [C, C], F32, kind="ExternalInput").ap()
    wkT = nc.dram_tensor("wkT", [C, C], F32, kind="ExternalInput").ap()
    wvT = nc.dram_tensor("wvT", [C, C], F32, kind="ExternalInput").ap()
    wpT = nc.dram_tensor("wpT", [C, C], F32, kind="ExternalInput").ap()
    biasqk = nc.dram_tensor("biasqk", [128, 1], F32, kind="ExternalInput").ap()
    bvrep = nc.dram_tensor("bvrep", [128, NH], F32, kind="ExternalInput").ap()
    bp2 = nc.dram_tensor("bp2", [128, NCI], F32, kind="ExternalInput").ap()

    out_part = nc.dram_tensor("out_part", [B, MLOC, C, T], F32,
                              kind="ExternalOutput").ap()
    a_part = nc.dram_tensor("a_part", [T, T], F32, kind="ExternalOutput").ap()
    if KDEBUG:
        dbg_xs = nc.dram_tensor("dbg_xs", [128, NCI * T], F32, kind="ExternalOutput").ap()
        dbg_qkraw = nc.dram_tensor("dbg_qkraw", [128, T], F32, kind="ExternalOutput").ap()
        dbg_rsin = nc.dram_tensor("dbg_rsin", [NH * 128, T], F32, kind="ExternalOutput").ap()
        dbg_st = nc.dram_tensor("dbg_st", [128, T], F32, kind="ExternalOutput").ap()
        dbg_wt = nc.dram_tensor("dbg_wt", [128, ST * T], F32, kind="ExternalOutput").ap()
        dbg_q = nc.dram_tensor("dbg_q", [D, T], F32, kind="ExternalOutput").ap()
        dbg_k = nc.dram_tensor("dbg_k", [D, T], F32, kind="ExternalOutput").ap()
        dbg_eT = nc.dram_tensor("dbg_eT", [128, T], F32, kind="ExternalOutput").ap()
        dbg_row = nc.dram_tensor("dbg_row", [1, T], F32, kind="ExternalOutput").ap()
        dbg_v = nc.dram_tensor("dbg_v", [128, C], F32, kind="ExternalOutput").ap()
        dbg_att = nc.dram_tensor("dbg_att", [128, T], F32, kind="ExternalOutput").ap()

    RG = [list(range(N_CORES))]

    with tile.TileContext(nc) as tc:
        with ExitStack() as stack:
            const = stack.enter_context(tc.tile_pool(name="const", bufs=1))
            data = stack.enter_context(tc.tile_pool(name="data", bufs=1))
            stage = stack.enter_context(tc.tile_pool(name="stage", bufs=2))
            soft = stack.enter_context(tc.tile_pool(name="soft", bufs=2))
            early = ExitStack()
            edata = early.enter_context(tc.tile_pool(name="edata", bufs=1))
            dram = stack.enter_context(tc.tile_pool(name="dram", bufs=1, space="DRAM"))
            ps_early = ExitStack()
            ps_qk = ps_early.enter_context(tc.tile_pool(name="ps_qk", bufs=2, space="PSUM"))
            ps_v = ps_early.enter_context(tc.tile_pool(name="ps_v", bufs=2, space="PSUM"))
            ps_sc = ps_early.enter_context(tc.tile_pool(name="ps_sc", bufs=1, space="PSUM"))
            ps_misc = ps_early.enter_context(tc.tile_pool(name="ps_misc", bufs=1, space="PSUM"))

            # ---------------- constants -----------------
            # fp16 weight tiles [128(ci), 256(co)] per k-tile (cast during DMA)
            wq16 = [const.tile([128, C], F16, name=f"wq16_{j}") for j in range(NCI)]
            wk16 = [const.tile([128, C], F16, name=f"wk16_{j}") for j in range(NCI)]
            wv16 = [const.tile([128, C], F16, name=f"wv16_{j}") for j in range(NCI)]
            wp16 = [const.tile([128, C], F16, name=f"wp16_{j}") for j in range(NCI)]
            for j in range(NCI):
                nc.gpsimd.dma_start(out=wq16[j], in_=wqT[j * 128:(j + 1) * 128, :])
                nc.gpsimd.dma_start(out=wk16[j], in_=wkT[j * 128:(j + 1) * 128, :])
                nc.gpsimd.dma_start(out=wv16[j], in_=wvT[j * 128:(j + 1) * 128, :])
                nc.gpsimd.dma_start(out=wp16[j], in_=wpT[j * 128:(j + 1) * 128, :])

            # per-partition affine for post-ReduceScatter qt/kt (host-packed):
            #   rows [0:64)=q (b0,b1): out = in/32 + bq ; rows [64:128)=k: out = in + 32*bk
            bias_qk = const.tile([128, 1], F32)
            nc.sync.dma_start(out=bias_qk, in_=biasqk)
            scale_qk = const.tile([128, 1], F32)
            nc.vector.memset(scale_qk[0:64, :], 1.0 / 32.0)
            nc.vector.memset(scale_qk[64:128, :], 1.0)

            bvr_sb = const.tile([128, NH], F32)
            nc.sync.dma_start(out=bvr_sb, in_=bvrep)
            bv_rep = [bvr_sb[:, n:n + 1] for n in range(NH)]
            bp_sb = const.tile([128, NCI], F32)
            nc.sync.dma_start(out=bp_sb, in_=bp2)
            bp_col = [bp_sb[:, g:g + 1] for g in range(NCI)]

            ones_col = const.tile([128, 1], F32)
            nc.vector.memset(ones_col, 1.0)
            ones_row = const.tile([1, 128], F32)
            nc.vector.memset(ones_row, 1.0)

            # ---------------- x partial sums (accumulating DMA) -------------
            # xs_sb[b][p, j*T + t] = sum_m x_sh[b, m, j*128+p, t]
            xs_sb = [edata.tile([128, NCI * T], F32, name=f"xs_sb{b}") for b in range(B)]
            for b in range(B):
                dst = xs_sb[b].rearrange("p (j t) -> p j t", j=NCI)
                for mh in range(MLOC):
                    src = x_sh[b, mh].rearrange("(j p) t -> p j t", p=128)
                    nc.gpsimd.dma_start(
                        out=dst, in_=src,
                        accum_op=(ALU.bypass if mh == 0 else ALU.add))

            # ------------- y loads (fp16 cast during DMA) + partial sums -----
            y16 = [[edata.tile([128, MLOC, T], F16, name=f"y16_{b}_{j}")
                    for j in range(NCI)] for b in range(B)]
            for b in range(B):
                for j in range(NCI):
                    nc.gpsimd.dma_start(
                        out=y16[b][j],
                        in_=y_sh[b, :, j * 128:(j + 1) * 128, :].rearrange(
                            "m p t -> p m t"))

            if KDEBUG:
                nc.sync.dma_start(out=dbg_xs, in_=xs_sb[0])

            ys_sb = [edata.tile([128, NCI * T], F32, name=f"ys_sb{b}") for b in range(B)]
            for b in range(B):
                for j in range(NCI):
                    tmp = stage.tile([128, 2, T], F32, name="ys_tmp")
                    nc.vector.tensor_add(
                        tmp, y16[b][j][:, 0:2, :], y16[b][j][:, 2:4, :])
                    nc.vector.tensor_add(
                        ys_sb[b][:, j * T:(j + 1) * T], tmp[:, 0, :], tmp[:, 1, :])

            xs16 = [edata.tile([128, NCI * T], F16, name=f"xs16_{b}") for b in range(B)]
            ys16 = [edata.tile([128, NCI * T], F16, name=f"ys16_{b}") for b in range(B)]
            for b in range(B):
                nc.vector.tensor_copy(xs16[b], xs_sb[b])
                nc.vector.tensor_copy(ys16[b], ys_sb[b])

            # ---------------- qt/kt projections -> ReduceScatter -------------
            # qk_psum[co, t] = sum_ci W^T[ci, co] * sums^T[ci, t]   (per b)
            rs_in = dram.tile([NH * 128, T], F32)
            rs_out = dram.tile([128, T], F32)
            rs_view = rs_in.rearrange("(n qk b d) t -> n qk b d t",
                                      n=NH, qk=2, b=B, d=D)
            for qk in range(2):
                w16 = wq16 if qk == 0 else wk16
                src16 = xs16 if qk == 0 else ys16
                for b in range(B):
                    for cot in range(NCI):
                        pqk = ps_qk.tile([128, T], F32, name="pqk")
                        for j in range(NCI):
                            nc.tensor.matmul(
                                pqk,
                                lhsT=w16[j][:, cot * 128:(cot + 1) * 128],
                                rhs=src16[b][:, j * T:(j + 1) * T],
                                start=(j == 0), stop=(j == NCI - 1))
                        st = stage.tile([128, T], F32, name="qk_st")
                        if (b + cot) % 2 == 0:
                            nc.scalar.copy(st, pqk)
                        else:
                            nc.vector.tensor_copy(st, pqk)
                        for blk in range(4):
                            eng = nc.sync if blk % 2 == 0 else nc.scalar
                            eng.dma_start(
                                out=rs_view[cot * 4 + blk, qk, b],
                                in_=st[blk * D:(blk + 1) * D, :])
                        if KDEBUG and qk == 0 and b == 0 and cot == 0:
                            nc.sync.dma_start(out=dbg_st, in_=st)

            if KDEBUG:
                nc.sync.dma_start(out=dbg_rsin, in_=rs_in)
            nc.gpsimd.collective_compute(
                "ReduceScatter", ALU.add, replica_groups=RG,
                ins=[rs_in.opt()], outs=[rs_out.opt()])

            # ---------------- V projection (overlaps the collectives) --------
            # v[s, co] per (b, mh):  lhsT = y^T[ci, s-block], rhs = Wv^T[ci, co]
            # stored as v_stk[b][s%128, sb, n, mh, d] so the PV stationary
            # (4 m-slices of one head) is a contiguous [128, 128] free slice
            v_stk = [data.tile([128, ST, NH, MLOC, D], F16, name=f"v_stk{b}")
                     for b in range(B)]
            for b in range(B):
                for mh in range(MLOC):
                    for sb in range(ST):
                        pv = ps_v.tile([128, C], F32, name="pv")
                        for j in range(NCI):
                            nc.tensor.matmul(
                                pv,
                                lhsT=y16[b][j][:, mh, sb * 128:(sb + 1) * 128],
                                rhs=wv16[j],
                                start=(j == 0), stop=(j == NCI - 1))
                        dst = v_stk[b][:, sb, :, mh, :]
                        if sb % 2 == 0:
                            nc.scalar.copy(dst, pv.rearrange("p (n d) -> p n d", d=D))
                        else:
                            nc.vector.tensor_copy(dst, pv.rearrange("p (n d) -> p n d", d=D))
                        if KDEBUG and b == 0 and mh == 0 and sb == 0:
                            vd = stage.tile([128, C], F32, name="vd")
                            nc.vector.tensor_copy(vd, pv)
                            nc.sync.dma_start(out=dbg_v, in_=vd)

            early.close()
            wpool = stack.enter_context(tc.tile_pool(name="wpool", bufs=9))
            opool = stack.enter_context(tc.tile_pool(name="opool", bufs=4))

            # ---------------- own-head scores + softmax ----------------------
            qk_raw = soft.tile([128, T], F32)
            nc.sync.dma_start(out=qk_raw, in_=rs_out)
            # split into base-partition-0 fp16 tiles with the affine applied
            q_t = [soft.tile([D, T], F16, name=f"q_t{b}") for b in range(B)]
            k_t = [soft.tile([D, T], F16, name=f"k_t{b}") for b in range(B)]
            AFI = AF.Identity
            for b in range(B):
                nc.scalar.activation(q_t[b], qk_raw[b * 32:(b + 1) * 32, :], AFI,
                                     bias=bias_qk[b * 32:(b + 1) * 32, :],
                                     scale=scale_qk[b * 32:(b + 1) * 32, :])
                nc.scalar.activation(k_t[b], qk_raw[64 + b * 32:64 + (b + 1) * 32, :], AFI,
                                     bias=bias_qk[64 + b * 32:64 + (b + 1) * 32, :],
                                     scale=scale_qk[64 + b * 32:64 + (b + 1) * 32, :])

            if KDEBUG:
                nc.sync.dma_start(out=dbg_qkraw, in_=qk_raw)
                q32d = stage.tile([D, T], F32, name="q32d")
                nc.vector.tensor_copy(q32d, q_t[0])
                nc.sync.dma_start(out=dbg_q, in_=q32d)
                k32d = stage.tile([D, T], F32, name="k32d")
                nc.vector.tensor_copy(k32d, k_t[0])
                nc.sync.dma_start(out=dbg_k, in_=k32d)

            ag_in = dram.tile([B * T, T], F16)
            ag_out = dram.tile([N_CORES * B * T, T], F16, addr_space="Shared")
            ag_in_view = ag_in.rearrange("(b sb p) t -> b p sb t", b=B, p=128)

            for b in range(B):
                # scores^T[s, t] = sum_d kt[d, s] qt[d, t]
                eT = [soft.tile([128, T], F32, name=f"eT{sb}", tag=f"eT{sb}")
                      for sb in range(ST)]
                for sb in range(ST):
                    psc = ps_sc.tile([128, T], F32, name="psc")
                    nc.tensor.matmul(psc,
                                     lhsT=k_t[b][:, sb * 128:(sb + 1) * 128],
                                     rhs=q_t[b], start=True, stop=True)
                    nc.scalar.activation(eT[sb], psc, AF.Exp)
                # rowsum over s (partition axis) via ones-matmul, then 1/x
                prow = ps_misc.tile([1, T], F32, name="prow")
                for sb in range(ST):
                    nc.tensor.matmul(prow, lhsT=ones_col, rhs=eT[sb],
                                     start=(sb == 0), stop=(sb == ST - 1))
                if KDEBUG and b == 0:
                    nc.sync.dma_start(out=dbg_eT, in_=eT[0])
                    rowd = stage.tile([1, T], F32, name="rowd")
                    nc.vector.tensor_copy(rowd, prow)
                    nc.sync.dma_start(out=dbg_row, in_=rowd)
                recip = soft.tile([1, T], F32, name="recip")
                nc.vector.reciprocal(recip, prow)
                # broadcast 1/rowsum to all partitions (K=1 fp32 matmul)
                pbc = ps_misc.tile([128, T], F32, name="pbc")
                nc.tensor.matmul(pbc, lhsT=ones_row, rhs=recip,
                                 start=True, stop=True)
                # normalize; keep fp32 for the `a` output (batch 0 only)
                wT16 = soft.tile([128, ST, T], F16, name="wT16")
                if b == 0:
                    wT32 = soft.tile([128, ST, T], F32, name="wT32", bufs=1)
                    for sb in range(ST):
                        nc.vector.tensor_mul(wT32[:, sb, :], eT[sb], pbc)
                    nc.sync.dma_start(
                        out=a_part.rearrange("(sb p) t -> p sb t", p=128),
                        in_=wT32)
                    nc.vector.tensor_copy(wT16, wT32)
                else:
                    for sb in range(ST):
                        nc.vector.tensor_mul(wT16[:, sb, :], eT[sb], pbc)
                nc.sync.dma_start(out=ag_in_view[b], in_=wT16)

            nc.gpsimd.collective_compute(
                "AllGather", ALU.bypass, replica_groups=RG,
                ins=[ag_in.opt()], outs=[ag_out.opt()])

            ps_early.close()
            ps_att = stack.enter_context(tc.tile_pool(name="ps_att", bufs=6, space="PSUM"))
            ps_out = stack.enter_context(tc.tile_pool(name="ps_out", bufs=2, space="PSUM"))

            # ---------------- PV + output projection -------------------------
            ag_view = ag_out.rearrange("(r b sb p) t -> r b p sb t",
                                       b=B, sb=ST, p=128)
            for b in range(B):
                w_t = []
                for n in range(NH):
                    wt = wpool.tile([128, ST, T], F16, name="w_t", tag="w_t")
                    nc.sync.dma_start(out=wt, in_=ag_view[n, b])
                    w_t.append(wt)
                    if KDEBUG and b == 0 and n == 2:
                        wtd = stage.tile([128, ST * T], F32, name="wtd", bufs=1)
                        nc.vector.tensor_copy(wtd, wt.rearrange("p s t -> p (s t)"))
                        nc.sync.dma_start(out=dbg_wt, in_=wtd)
                # att^T[(mh d), t] per head: full-128 stationary, shared rhs
                att_big = [opool.tile([128, T], F16, name=f"att_big{n}",
                                      tag=f"att_big{n}", bufs=2) for n in range(NH)]
                for n in range(NH):
                    patt = ps_att.tile([128, T], F32, name="patt")
                    for sb in range(ST):
                        nc.tensor.matmul(
                            patt,
                            lhsT=v_stk[b][:, sb, n, :, :],
                            rhs=w_t[n][:, sb, :],
                            start=(sb == 0), stop=(sb == ST - 1))
                    # + bv (sum_s w = 1 after softmax normalization)
                    nc.scalar.activation(att_big[n], patt, AFI,
                                         bias=bv_rep[n], scale=1.0)
                if KDEBUG and b == 0:
                    attd = stage.tile([128, T], F32, name="attd")
                    nc.vector.tensor_copy(attd, att_big[0])
                    nc.sync.dma_start(out=dbg_att, in_=attd)
                for mh in range(MLOC):
                    # partition remap [(mh d) of head n] -> [(n d)] via SBUF DMAs
                    attm = [opool.tile([128, T], F16, name="attm", tag="attm")
                            for _ in range(NCI)]
                    for n in range(NH):
                        eng = (nc.sync, nc.scalar, nc.gpsimd)[n % 3]
                        eng.dma_start(
                            out=attm[n // 4][(n % 4) * D:(n % 4 + 1) * D, :],
                            in_=att_big[n][mh * D:(mh + 1) * D, :])
                    o_sb = opool.tile([128, NCI, T], F32, name="o_sb", tag="o_sb")
                    for cot in range(NCI):
                        pout = ps_out.tile([128, T], F32, name="pout")
                        for ct in range(NCI):
                            nc.tensor.matmul(
                                pout,
                                lhsT=wp16[ct][:, cot * 128:(cot + 1) * 128],
                                rhs=attm[ct],
                                start=(ct == 0), stop=(ct == NCI - 1),
                                skip_group_check=True)
                        nc.scalar.activation(o_sb[:, cot, :], pout, AFI,
                                             bias=bp_col[cot], scale=1.0)
                    nc.sync.dma_start(
                        out=out_part[b, mh].rearrange("(cot p) t -> p cot t", p=128),
                        in_=o_sb)
    nc.compile()
    return nc


def _get_nc():
    if "nc" not in _CACHE:
        _CACHE["nc"] = _build()
    return _CACHE["nc"]


def _make_in_maps(inputs):
    return _prep(**inputs)


def _prep(x, y, Wq, bq, Wk, bk, Wv, bv, Wp, bp):
    x = np.ascontiguousarray(np.asarray(x, dtype=np.float32))
    y = np.ascontiguousarray(np.asarray(y, dtype=np.float32))
    xT = np.ascontiguousarray(x.transpose(0, 1, 3, 2))
    yT = np.ascontiguousarray(y.transpose(0, 1, 3, 2))
    wqT = np.ascontiguousarray(np.asarray(Wq, np.float32).T)
    wkT = np.ascontiguousarray(np.asarray(Wk, np.float32).T)
    wvT = np.ascontiguousarray(np.asarray(Wv, np.float32).T)
    wpT = np.ascontiguousarray(np.asarray(Wp, np.float32).T)
    bq = np.asarray(bq, np.float32)
    bk = np.asarray(bk, np.float32)
    bv = np.asarray(bv, np.float32)
    bp = np.asarray(bp, np.float32)
    # host-side bias packing: pure indexing/tiling (layout prep only)
    bvrep = np.ascontiguousarray(
        np.tile(bv.reshape(NH, D).T[None, :, :], (MLOC, 1, 1))
        .reshape(128, NH))          # [mh*32+d, n] = bv[n*32+d]
    bp2 = np.ascontiguousarray(bp.reshape(NCI, 128).T)  # [p, g] = bp[g*128+p]

    in_maps = []
    for i in range(N_CORES):
        biasqk = np.concatenate(
            [bq[i * D:(i + 1) * D], bq[i * D:(i + 1) * D],
             M * bk[i * D:(i + 1) * D], M * bk[i * D:(i + 1) * D]]
        ).reshape(128, 1).astype(np.float32)
        in_maps.append(dict(
            x_sh=np.ascontiguousarray(xT[:, i * MLOC:(i + 1) * MLOC]),
            y_sh=np.ascontiguousarray(yT[:, i * MLOC:(i + 1) * MLOC]),
            wqT=wqT, wkT=wkT, wvT=wvT, wpT=wpT,
            biasqk=biasqk, bvrep=bvrep, bp2=bp2,
        ))

    return in_maps


def kernel(x, y, Wq, bq, Wk, bk, Wv, bv, Wp, bp):
    import concourse.bass_utils as bass_utils

    nc = _get_nc()
    in_maps = _prep(x, y, Wq, bq, Wk, bk, Wv, bv, Wp, bp)
    res = bass_utils.run_bass_kernel_spmd(nc, in_maps, core_ids=list(range(N_CORES)))

    out = np.empty((B, M, T, C), dtype=np.float32)
    a = np.empty((NH, T, T), dtype=np.float32)
    for i in range(N_CORES):
        op = res.results[i]["out_part"]          # [B, MLOC, C, T]
        out[:, i * MLOC:(i + 1) * MLOC] = op.transpose(0, 1, 3, 2)
        a[i] = res.results[i]["a_part"].T
    return out, a
